# revision 4
# baseline (speedup 1.0000x reference)
"""Trainium2 Bass kernel for nn_DecoderLayer (self-attn + cross-attn + FFN layer).

Sharding: 8 cores = (batch, query-half). Core c handles batch c//2 and query rows
[512*(c%2), 512*(c%2)+512). Each core computes the full layer for its 512 query
tokens; K/V work over the full 1024 key tokens is duplicated between the two
cores of a batch (cheaper than collectives at this size).

On-chip layout: transposed activations (features on partitions, tokens free), so
projections chain with no on-chip transposes. Attention uses the transposed-
scores formulation: scores^T[k,q] = K^T-chunk (stationary) @ Q^T (moving), plus
an identity-matmul accumulating the host-premultiplied (-1e9) mask into the same
PSUM; exp runs on the scalar engine straight out of PSUM; the AV matmul consumes
probs^T directly with V stored tokens-on-partitions, and a fused ones column in
the V tile yields the softmax denominators in the same matmul. All matmuls run
as float32r (full-rate fp32, ~2e-4 rel err). LayerNorm reduces across partitions
via ones-vector matmuls; per-token scale/bias broadcast via K=1 ones matmuls.

Host side: pre-transposes per-core activations (rotating so each core's own
query block sits in columns [0:512) — the key axis is permutation-invariant as
long as the mask rows are permuted identically), pre-packs weight tiles, and
transposes the returned out^T back.
"""

import numpy as np

B, T, D, H, DH, FFN = 4, 1024, 1024, 16, 64, 4096
EPS = 1e-5
P = 128
ND = D // P          # 8 contraction chunks over D
NF = FFN // P        # 32 ffn chunks
TQ = 512             # query tokens per core
NKC = T // P         # 8 key chunks
NHP = H // 2         # 8 head pairs

_CACHE = {}


def _split_waits(nc, maxw=1):
    """Walrus in this toolchain encodes at most one sem-wait per TPB
    instruction; distribute excess waits onto preceding same-engine NOPs."""
    import bass_rust
    import concourse.mybir as mybir

    for bbw in nc.main_func.blocks:
        insts = bbw.instructions
        out = []
        changed = False
        for inst in insts:
            si = inst.sync_info
            waits = list(si.on_wait or []) if si is not None else []
            if len(waits) > maxw:
                changed = True
                extra = waits[: len(waits) - maxw]
                si.on_wait = waits[len(waits) - maxw :]
                for i in range(0, len(extra), maxw):
                    nop = mybir.InstNoOp(
                        name=nc.get_next_instruction_name(),
                        ins=[],
                        outs=[],
                        engine=inst.engine,
                        sync_info=bass_rust.SyncInfo(
                            on_wait=extra[i : i + maxw], on_update=[]
                        ),
                    )
                    nc.register_instruction(nop, overwrite=True)
                    out.append(nop)
            out.append(inst)
        if changed:
            bbw.instructions[:] = out


def _build_decoder():
    import concourse.bass as bass
    import concourse.mybir as mybir
    import concourse.tile as tile
    from contextlib import ExitStack

    F32 = mybir.dt.float32
    F32R = mybir.dt.float32r
    AF = mybir.ActivationFunctionType
    ALU = mybir.AluOpType

    nc = bass.Bass()

    def din(name, shape):
        return nc.dram_tensor(name, shape, F32, kind="ExternalInput")

    tT = din("tT", [D, T])        # target^T, own query block first
    sT = din("sT", [D, T])        # source^T
    mT = din("mT", [T, TQ])       # mask bias^T (-1e9 where masked), rows permuted
    hident = din("hident", [P, P])
    hones = din("hones", [P, P])
    wq1 = din("wq1", [ND, ND, P, P]);  bq1 = din("bq1", [D])
    wk1 = din("wk1", [ND, ND, P, P]);  bk1 = din("bk1", [D])
    wv1 = din("wv1", [D, D]);          bv1 = din("bv1", [1, D])
    wo1 = din("wo1", [ND, ND, P, P]);  bo1 = din("bo1", [D])
    wq2 = din("wq2", [ND, ND, P, P]);  bq2 = din("bq2", [D])
    wk2 = din("wk2", [ND, ND, P, P]);  bk2 = din("bk2", [D])
    wv2 = din("wv2", [D, D]);          bv2 = din("bv2", [1, D])
    wo2 = din("wo2", [ND, ND, P, P]);  bo2 = din("bo2", [D])
    w1r = din("w1r", [NF, ND, P, P]);  b1 = din("b1", [FFN])
    w2r = din("w2r", [ND, NF, P, P]);  b2 = din("b2", [D])
    ln_g = din("ln_g", [3, D])
    ln_b = din("ln_b", [3, D])
    outT = nc.dram_tensor("outT", [D, TQ], F32, kind="ExternalOutput")

    with tile.TileContext(nc) as tc, ExitStack() as glob:
        consts = glob.enter_context(tc.tile_pool(name="consts", bufs=1))
        probsp = glob.enter_context(tc.tile_pool(name="probs", bufs=2))
        smallp = glob.enter_context(tc.tile_pool(name="smalls", bufs=1))
        ctxp = glob.enter_context(tc.tile_pool(name="ctx", bufs=8))
        residp = glob.enter_context(tc.tile_pool(name="resid", bufs=8))

        ident = consts.tile([P, P], F32R)
        nc.sync.dma_start(ident[:], hident[:].bitcast(F32R))
        ones_c = consts.tile([1, P], F32R)
        nc.sync.dma_start(ones_c[:], hones[0:1, :].bitcast(F32R))
        ones_r = consts.tile([P, 1], F32R)
        nc.sync.dma_start(ones_r[:], hones[:, 0:1].bitcast(F32R))
        ones16 = consts.tile([P, 16], F32)
        nc.sync.dma_start(ones16[:], hones[:, 0:16])
        eps_t = consts.tile([1, 1], F32)
        nc.vector.memset(eps_t, EPS)
        lng = consts.tile([P, 3, ND], F32)
        nc.sync.dma_start(lng[:], ln_g.rearrange("l (c p) -> p l c", p=P))
        lnb = consts.tile([P, 3, ND], F32)
        nc.sync.dma_start(lnb[:], ln_b.rearrange("l (c p) -> p l c", p=P))

        def f32(ap):
            return ap.bitcast(F32)

        def load_rows(pool, dram, ncols, tag):
            tiles = []
            for c in range(dram.shape[0] // P):
                t_ = pool.tile([P, ncols], F32R, tag=tag)
                nc.sync.dma_start(t_[:], dram[c * P : (c + 1) * P, :].bitcast(F32R))
                tiles.append(t_)
            return tiles

        def proj_chunk(wr, bvec, jc, x_tiles, cols, out_pool, out_tag, wpool,
                       pspool, pstag="pps", func=AF.Identity, out_dtype=F32R):
            """One output-feature chunk jc of out^T = func(W.T @ X^T + b)."""
            ndc = wr.shape[1]
            wsl = wpool.tile([P, ndc, P], F32R, tag="wsl")
            nc.sync.dma_start(wsl[:],
                              wr[jc].rearrange("c p m -> p c m").bitcast(F32R))
            bt = wpool.tile([P, 1], F32, tag="bt")
            nc.sync.dma_start(bt[:], bvec[jc * P : (jc + 1) * P, None])
            outs = []
            for c0, cn in cols:
                ps = pspool.tile([P, 512], F32, tag=pstag)
                for dc in range(ndc):
                    nc.tensor.matmul(ps[:], wsl[:, dc, :],
                                     x_tiles[dc][:, c0 : c0 + cn],
                                     start=(dc == 0), stop=(dc == ndc - 1))
                o = out_pool.tile([P, cn], out_dtype, tag=out_tag)
                nc.scalar.activation(out=o[:], in_=ps[:, 0:cn], func=func,
                                     bias=bt[:], scale=1.0)
                outs.append(o)
            return outs

        def proj_tokens(wv, bv, x_tiles, vpool, vwp, pspool):
            """V with fused ones column, tokens on partitions:
            vext[sc] [P, H, DH+1]."""
            vtiles = []
            for sc in range(NKC):
                vt = vpool.tile([P, H, DH + 1], F32R, tag="vext")
                nc.vector.tensor_copy(out=vt[:, :, DH : DH + 1],
                                      in_=ones16[:, :, None])
                vtiles.append(vt)
            for q in range(4):  # quarter = 256 features = 4 heads
                wslab = vwp.tile([P, ND, 256], F32R, tag="vwsl")
                nc.sync.dma_start(
                    wslab[:],
                    wv[:, q * 256 : (q + 1) * 256].rearrange(
                        "(c p) n -> p c n", p=P).bitcast(F32R))
                bvt = vwp.tile([1, 256], F32R, tag="vbias")
                nc.sync.dma_start(bvt[:],
                                  bv[:, q * 256 : (q + 1) * 256].bitcast(F32R))
                for sc in range(NKC):
                    ps = pspool.tile([P, 256], F32, tag="vps")
                    for dc in range(ND):
                        nc.tensor.matmul(ps[:],
                                         x_tiles[dc][:, sc * P : (sc + 1) * P],
                                         wslab[:, dc, :],
                                         start=(dc == 0), stop=False)
                    nc.tensor.matmul(ps[:], ones_c[:], bvt[:],
                                     start=False, stop=True)
                    dst = vtiles[sc][:, q * 4 : (q + 1) * 4, 0:DH]
                    nc.scalar.activation(
                        out=dst, in_=ps[:].rearrange("p (h d) -> p h d", h=4),
                        func=AF.Copy)
            return vtiles

        def attention(qt_tiles, kt_producer, v_tiles, mask_tiles, pspools):
            """qt_tiles: 8 [P, TQ] (2 heads per tile); kt_producer(hp) -> 2
            half tiles [P, 512]; v_tiles: 8 [P, H, DH+1]; mask_tiles or None."""
            scp, ctxps, bcps = pspools
            ctx_tiles = []
            for hp in range(NHP):
                kt_h = kt_producer(hp)
                ctx_t = ctxp.tile([P, TQ], F32R, tag="ctxT")
                ctx_tiles.append(ctx_t)
                qa = qt_tiles[hp][0:64, :]
                qb = qt_tiles[hp][64:128, :]
                psc_a = ctxps.tile([65, TQ], F32, tag="ctxps_a")
                psc_b = ctxps.tile([65, TQ], F32, tag="ctxps_b")
                for kc in range(NKC):
                    kt_t = kt_h[kc // 4]
                    kcol = (kc % 4) * P
                    psA = scp.tile([P, TQ], F32, tag="scA")
                    psB = scp.tile([P, TQ], F32, tag="scB")
                    ka = kt_t[0:64, kcol : kcol + P]
                    kb = kt_t[64:128, kcol : kcol + P]
                    has_mask = mask_tiles is not None
                    nc.tensor.matmul(psA[:], ka, qa, start=True,
                                     stop=not has_mask, tile_position=(0, 0))
                    nc.tensor.matmul(psB[:], kb, qb, start=True,
                                     stop=not has_mask, tile_position=(64, 0))
                    if has_mask:
                        m = mask_tiles[kc]
                        nc.tensor.matmul(psA[:], ident[:], m[:],
                                         start=False, stop=True)
                        nc.tensor.matmul(psB[:], ident[:], m[:],
                                         start=False, stop=True)
                    pA = probsp.tile([P, TQ], F32R, tag="prA")
                    pB = probsp.tile([P, TQ], F32R, tag="prB")
                    nc.scalar.activation(out=pA[:], in_=psA[:], func=AF.Exp,
                                         scale=0.125)
                    nc.scalar.activation(out=pB[:], in_=psB[:], func=AF.Exp,
                                         scale=0.125)
                    va = v_tiles[kc][:, 2 * hp, :]
                    vb = v_tiles[kc][:, 2 * hp + 1, :]
                    nc.tensor.matmul(psc_a[:], va, pA[:],
                                     start=(kc == 0), stop=(kc == NKC - 1))
                    nc.tensor.matmul(psc_b[:], vb, pB[:],
                                     start=(kc == 0), stop=(kc == NKC - 1))
                for par, psc in ((0, psc_a), (1, psc_b)):
                    rf = smallp.tile([1, TQ], F32, tag="recf")
                    nc.vector.reciprocal(out=rf[:], in_=psc[64:65, :])
                    rr = smallp.tile([1, TQ], F32R, tag="recr")
                    nc.vector.tensor_copy(out=rr[:], in_=rf[:])
                    psb = bcps.tile([64, TQ], F32, tag="bc")
                    nc.tensor.matmul(psb[:], ones_c[:, 0:64], rr[:],
                                     start=True, stop=True)
                    csb = probsp.tile([64, TQ], F32, tag="csb")
                    nc.scalar.activation(out=csb[:], in_=psc[0:64, :],
                                         func=AF.Copy)
                    nc.vector.tensor_tensor(
                        out=ctx_t[par * 64 : (par + 1) * 64, :], in0=csb[:],
                        in1=psb[:], op=ALU.mult)
            return ctx_tiles

        def layer_norm(pre_tiles, ln_idx, out_pool, out_tag, pools,
                       out_dtype=F32R):
            """LN across the partition (feature) axis of 8 F32R [P, TQ] tiles."""
            tmpp, statps, bcps = pools
            sq_tiles = []
            for dc in range(ND):
                sq = tmpp.tile([P, TQ], F32R, tag="lnsq")
                nc.vector.tensor_tensor(out=sq[:], in0=f32(pre_tiles[dc][:]),
                                        in1=f32(pre_tiles[dc][:]), op=ALU.mult)
                sq_tiles.append(sq)
            ps1 = statps.tile([1, TQ], F32, tag="lns1")
            ps2 = statps.tile([1, TQ], F32, tag="lns2")
            for dc in range(ND):
                nc.tensor.matmul(ps1[:], ones_r[:], pre_tiles[dc][:],
                                 start=(dc == 0), stop=(dc == ND - 1))
            for dc in range(ND):
                nc.tensor.matmul(ps2[:], ones_r[:], sq_tiles[dc][:],
                                 start=(dc == 0), stop=(dc == ND - 1))
            mu = smallp.tile([1, TQ], F32, tag="lnmu")
            nc.vector.tensor_scalar_mul(out=mu[:], in0=ps1[:], scalar1=1.0 / D)
            ex2 = smallp.tile([1, TQ], F32, tag="lnex2")
            nc.vector.tensor_scalar_mul(out=ex2[:], in0=ps2[:], scalar1=1.0 / D)
            var = smallp.tile([1, TQ], F32, tag="lnvar")
            nc.vector.tensor_tensor(out=var[:], in0=mu[:], in1=mu[:], op=ALU.mult)
            nc.vector.tensor_tensor(out=var[:], in0=ex2[:], in1=var[:],
                                    op=ALU.subtract)
            sd = smallp.tile([1, TQ], F32, tag="lnsd")
            nc.scalar.activation(out=sd[:], in_=var[:], func=AF.Sqrt,
                                 bias=eps_t[:], scale=1.0)
            rsf = smallp.tile([1, TQ], F32, tag="lnrsf")
            nc.vector.reciprocal(out=rsf[:], in_=sd[:])
            cmul = smallp.tile([1, TQ], F32, tag="lncm")
            nc.vector.tensor_tensor(out=cmul[:], in0=mu[:], in1=rsf[:],
                                    op=ALU.mult)
            cneg = smallp.tile([1, TQ], F32R, tag="lncn")
            nc.vector.tensor_scalar_mul(out=cneg[:], in0=cmul[:], scalar1=-1.0)
            rsig = smallp.tile([1, TQ], F32R, tag="lnrs")
            nc.vector.tensor_copy(out=rsig[:], in_=rsf[:])
            psa = bcps.tile([P, TQ], F32, tag="lnbca")
            nc.tensor.matmul(psa[:], ones_c[:], rsig[:], start=True, stop=True)
            psc = bcps.tile([P, TQ], F32, tag="lnbcc")
            nc.tensor.matmul(psc[:], ones_c[:], cneg[:], start=True, stop=True)
            out_tiles = []
            for dc in range(ND):
                t1 = tmpp.tile([P, TQ], F32, tag="lnt1")
                nc.vector.tensor_tensor(out=t1[:], in0=f32(pre_tiles[dc][:]),
                                        in1=psa[:], op=ALU.mult)
                t2 = tmpp.tile([P, TQ], F32, tag="lnt2")
                nc.vector.tensor_tensor(out=t2[:], in0=t1[:], in1=psc[:],
                                        op=ALU.add)
                o = out_pool.tile([P, TQ], out_dtype, tag=out_tag)
                nc.vector.tensor_scalar(
                    out=o[:], in0=t2[:],
                    scalar1=lng[:, ln_idx, dc : dc + 1],
                    scalar2=lnb[:, ln_idx, dc : dc + 1],
                    op0=ALU.mult, op1=ALU.add)
                out_tiles.append(o)
            return out_tiles

        def oproj_resid_ln(wo, bo, ctx_tiles, resid_f32_aps, ln_idx, out_tag,
                           stack):
            wpool = stack.enter_context(tc.tile_pool(name="owp", bufs=2))
            pps = stack.enter_context(tc.tile_pool(name="ops", bufs=2,
                                                   space="PSUM"))
            tmpp = stack.enter_context(tc.tile_pool(name="otmp", bufs=3))
            statps = stack.enter_context(
                tc.tile_pool(name="olnst", bufs=1, space="PSUM"))
            bcps = stack.enter_context(
                tc.tile_pool(name="olnbc", bufs=1, space="PSUM"))
            prep = stack.enter_context(tc.tile_pool(name="opre", bufs=8))
            pre_tiles = []
            for jc in range(ND):
                (osb,) = proj_chunk(wo, bo, jc, ctx_tiles, [(0, TQ)], tmpp,
                                    "osb", wpool, pps, out_dtype=F32)
                pre = prep.tile([P, TQ], F32R, tag="pre")
                nc.vector.tensor_tensor(out=pre[:], in0=osb[:],
                                        in1=resid_f32_aps[jc], op=ALU.add)
                pre_tiles.append(pre)
            return layer_norm(pre_tiles, ln_idx, residp, out_tag,
                              (tmpp, statps, bcps))

        # ================= phases =================
        with ExitStack() as ph_t:
            ttp = ph_t.enter_context(tc.tile_pool(name="tTp", bufs=8))
            tT_tiles = load_rows(ttp, tT, T, "tT")

            with ExitStack() as ph_sa:
                qtp = ph_sa.enter_context(tc.tile_pool(name="qtp", bufs=8))
                vxp = ph_sa.enter_context(tc.tile_pool(name="vxp", bufs=8))
                wp = ph_sa.enter_context(tc.tile_pool(name="saw", bufs=2))
                with ExitStack() as ph_p1:
                    pp = ph_p1.enter_context(
                        tc.tile_pool(name="p1ps", bufs=4, space="PSUM"))
                    qt_tiles = []
                    for jc in range(ND):
                        (q_,) = proj_chunk(wq1, bq1, jc, tT_tiles, [(0, TQ)],
                                           qtp, "qt", wp, pp)
                        qt_tiles.append(q_)
                    vwp = ph_p1.enter_context(tc.tile_pool(name="vwp", bufs=1))
                    v_tiles = proj_tokens(wv1, bv1, tT_tiles, vxp, vwp, pp)

                with ExitStack() as ph_a1:
                    mtp = ph_a1.enter_context(tc.tile_pool(name="mtp", bufs=8))
                    m_tiles = load_rows(mtp, mT, TQ, "mt")
                    ktp = ph_a1.enter_context(tc.tile_pool(name="ktp", bufs=3))
                    ktps = ph_a1.enter_context(
                        tc.tile_pool(name="ktps", bufs=1, space="PSUM"))
                    scp = ph_a1.enter_context(
                        tc.tile_pool(name="scps", bufs=2, space="PSUM"))
                    ctxps = ph_a1.enter_context(
                        tc.tile_pool(name="ctxps", bufs=1, space="PSUM"))
                    bcps = ph_a1.enter_context(
                        tc.tile_pool(name="bcps", bufs=1, space="PSUM"))

                    def kt1_producer(hp):
                        return proj_chunk(wk1, bk1, hp, tT_tiles,
                                          [(0, 512), (512, 512)], ktp, "kt",
                                          wp, ktps, pstag="ktps")

                    ctx1 = attention(qt_tiles, kt1_producer, v_tiles, m_tiles,
                                     (scp, ctxps, bcps))

            with ExitStack() as ph_o1:
                x_tiles = oproj_resid_ln(
                    wo1, bo1, ctx1, [f32(t_[:, 0:TQ]) for t_ in tT_tiles], 0,
                    "resid", ph_o1)

        with ExitStack() as ph_s:
            stp = ph_s.enter_context(tc.tile_pool(name="sTp", bufs=8))
            sT_tiles = load_rows(stp, sT, T, "sT")

            with ExitStack() as ph_ca:
                qtp2 = ph_ca.enter_context(tc.tile_pool(name="qtp2", bufs=8))
                vxp2 = ph_ca.enter_context(tc.tile_pool(name="vxp2", bufs=8))
                wp2 = ph_ca.enter_context(tc.tile_pool(name="caw", bufs=2))
                with ExitStack() as ph_p2:
                    pp = ph_p2.enter_context(
                        tc.tile_pool(name="p2ps", bufs=4, space="PSUM"))
                    qt2 = []
                    for jc in range(ND):
                        (q_,) = proj_chunk(wq2, bq2, jc, x_tiles, [(0, TQ)],
                                           qtp2, "qt2", wp2, pp)
                        qt2.append(q_)
                    vwp2 = ph_p2.enter_context(tc.tile_pool(name="vwp2", bufs=1))
                    v2 = proj_tokens(wv2, bv2, sT_tiles, vxp2, vwp2, pp)

                with ExitStack() as ph_a2:
                    ktp2 = ph_a2.enter_context(tc.tile_pool(name="ktp2", bufs=3))
                    ktps2 = ph_a2.enter_context(
                        tc.tile_pool(name="ktps2", bufs=1, space="PSUM"))
                    scp = ph_a2.enter_context(
                        tc.tile_pool(name="scps2", bufs=2, space="PSUM"))
                    ctxps = ph_a2.enter_context(
                        tc.tile_pool(name="ctxps2", bufs=1, space="PSUM"))
                    bcps = ph_a2.enter_context(
                        tc.tile_pool(name="bcps2", bufs=1, space="PSUM"))

                    def kt2_producer(hp):
                        return proj_chunk(wk2, bk2, hp, sT_tiles,
                                          [(0, 512), (512, 512)], ktp2, "kt2",
                                          wp2, ktps2, pstag="ktps2")

                    ctx2 = attention(qt2, kt2_producer, v2, None,
                                     (scp, ctxps, bcps))

            with ExitStack() as ph_o2:
                y_tiles = oproj_resid_ln(
                    wo2, bo2, ctx2, [f32(t_[:]) for t_ in x_tiles], 1, "resid",
                    ph_o2)

        # ---- FFN + LN3 ----
        with ExitStack() as ph_f:
            hpool = ph_f.enter_context(tc.tile_pool(name="hp", bufs=12))
            faccp = ph_f.enter_context(tc.tile_pool(name="facc", bufs=8))
            w1p = ph_f.enter_context(tc.tile_pool(name="w1p", bufs=2))
            w2p = ph_f.enter_context(tc.tile_pool(name="w2p", bufs=2))
            b2p = ph_f.enter_context(tc.tile_pool(name="b2p", bufs=8))
            ps1p = ph_f.enter_context(
                tc.tile_pool(name="fps1", bufs=2, space="PSUM"))
            ps2p = ph_f.enter_context(
                tc.tile_pool(name="fps2", bufs=2, space="PSUM"))
            facc = [None] * ND
            b2t = []
            for jc in range(ND):
                bt = b2p.tile([P, 1], F32, tag="b2t")
                nc.sync.dma_start(bt[:], b2[jc * P : (jc + 1) * P, None])
                b2t.append(bt)
            for g in range(4):
                h_tiles = []
                for kl in range(8):
                    k = g * 8 + kl
                    wsl = w1p.tile([P, ND, P], F32R, tag="w1sl")
                    nc.sync.dma_start(
                        wsl[:], w1r[k].rearrange("c p m -> p c m").bitcast(F32R))
                    bt = w1p.tile([P, 1], F32, tag="b1t")
                    nc.sync.dma_start(bt[:], b1[k * P : (k + 1) * P, None])
                    ps = ps1p.tile([P, TQ], F32, tag="w1ps")
                    for dc in range(ND):
                        nc.tensor.matmul(ps[:], wsl[:, dc, :], y_tiles[dc][:],
                                         start=(dc == 0), stop=(dc == ND - 1))
                    ht = hpool.tile([P, TQ], F32R, tag="h")
                    nc.scalar.activation(out=ht[:], in_=ps[:], func=AF.Relu,
                                         bias=bt[:], scale=1.0)
                    h_tiles.append(ht)
                for jc in range(ND):
                    wsl2 = w2p.tile([P, 8, P], F32R, tag="w2sl")
                    nc.sync.dma_start(
                        wsl2[:],
                        w2r[jc, g * 8 : (g + 1) * 8].rearrange(
                            "c p m -> p c m").bitcast(F32R))
                    ps = ps2p.tile([P, TQ], F32, tag="w2ps")
                    for dcl in range(8):
                        nc.tensor.matmul(ps[:], wsl2[:, dcl, :],
                                         h_tiles[dcl][:],
                                         start=(dcl == 0), stop=(dcl == 7))
                    if g == 0:
                        fa = faccp.tile([P, TQ], F32, tag="fa")
                        nc.scalar.activation(out=fa[:], in_=ps[:],
                                             func=AF.Identity,
                                             bias=b2t[jc][:], scale=1.0)
                        facc[jc] = fa
                    else:
                        nc.vector.tensor_tensor(out=facc[jc][:],
                                                in0=facc[jc][:], in1=ps[:],
                                                op=ALU.add)
            with ExitStack() as ph_l3:
                tmpp = ph_l3.enter_context(tc.tile_pool(name="l3tmp", bufs=3))
                statps = ph_l3.enter_context(
                    tc.tile_pool(name="l3st", bufs=1, space="PSUM"))
                bcps2 = ph_l3.enter_context(
                    tc.tile_pool(name="l3bc", bufs=1, space="PSUM"))
                prep = ph_l3.enter_context(tc.tile_pool(name="l3pre", bufs=8))
                outp = ph_l3.enter_context(tc.tile_pool(name="outp", bufs=3))
                pre3 = []
                for jc in range(ND):
                    pre = prep.tile([P, TQ], F32R, tag="pre3")
                    nc.vector.tensor_tensor(out=pre[:], in0=facc[jc][:],
                                            in1=f32(y_tiles[jc][:]),
                                            op=ALU.add)
                    pre3.append(pre)
                out_tiles = layer_norm(pre3, 2, outp, "out",
                                       (tmpp, statps, bcps2), out_dtype=F32)
                for jc in range(ND):
                    nc.sync.dma_start(outT[jc * P : (jc + 1) * P, :],
                                      out_tiles[jc][:])

    _split_waits(nc)
    return nc


def _host_pack(params):
    def pk(w):
        w = np.ascontiguousarray(np.asarray(w, dtype=np.float32))
        ndc = w.shape[0] // P
        njc = w.shape[1] // P
        return np.ascontiguousarray(
            w.reshape(ndc, P, njc, P).transpose(2, 0, 1, 3))

    p = params
    return {
        "wq1": pk(p["sa_q_W"]), "bq1": np.asarray(p["sa_q_b"], np.float32),
        "wk1": pk(p["sa_k_W"]), "bk1": np.asarray(p["sa_k_b"], np.float32),
        "wv1": np.ascontiguousarray(np.asarray(p["sa_v_W"], np.float32)),
        "bv1": np.asarray(p["sa_v_b"], np.float32).reshape(1, D),
        "wo1": pk(p["sa_o_W"]), "bo1": np.asarray(p["sa_o_b"], np.float32),
        "wq2": pk(p["ca_q_W"]), "bq2": np.asarray(p["ca_q_b"], np.float32),
        "wk2": pk(p["ca_k_W"]), "bk2": np.asarray(p["ca_k_b"], np.float32),
        "wv2": np.ascontiguousarray(np.asarray(p["ca_v_W"], np.float32)),
        "bv2": np.asarray(p["ca_v_b"], np.float32).reshape(1, D),
        "wo2": pk(p["ca_o_W"]), "bo2": np.asarray(p["ca_o_b"], np.float32),
        "w1r": pk(p["ffn_W1"]), "b1": np.asarray(p["ffn_b1"], np.float32),
        "w2r": pk(p["ffn_W2"]), "b2": np.asarray(p["ffn_b2"], np.float32),
        "ln_g": np.stack([np.asarray(p["ln1_g"], np.float32),
                          np.asarray(p["ln2_g"], np.float32),
                          np.asarray(p["ln3_g"], np.float32)]),
        "ln_b": np.stack([np.asarray(p["ln1_b"], np.float32),
                          np.asarray(p["ln2_b"], np.float32),
                          np.asarray(p["ln3_b"], np.float32)]),
        "hident": np.eye(P, dtype=np.float32),
        "hones": np.ones((P, P), dtype=np.float32),
    }


def kernel(target, source, pad_masked, params):
    from concourse.bass_utils import run_bass_kernel_spmd

    target = np.asarray(target, dtype=np.float32)
    source = np.asarray(source, dtype=np.float32)
    mask = np.asarray(pad_masked)

    wshared = _host_pack(params)

    in_maps = []
    for c in range(8):
        b, half = divmod(c, 2)
        q0 = half * TQ
        perm = np.concatenate([np.arange(q0, q0 + TQ),
                               np.arange(0, q0),
                               np.arange(q0 + TQ, T)]).astype(np.int64)
        m = dict(wshared)
        tTb = target[b].T  # [D, T]
        m["tT"] = np.ascontiguousarray(tTb[:, perm])
        m["sT"] = np.ascontiguousarray(source[b].T)
        mrows = (mask[b, q0 : q0 + TQ, :].astype(np.float32)
                 * np.float32(-1e9)).T  # [T keys, TQ]
        m["mT"] = np.ascontiguousarray(mrows[perm, :])
        in_maps.append(m)

    if "nc" not in _CACHE:
        _CACHE["nc"] = _build_decoder()
    res = run_bass_kernel_spmd(_CACHE["nc"], in_maps, core_ids=list(range(8)))

    out = np.empty((B, T, D), dtype=np.float32)
    for c in range(8):
        b, half = divmod(c, 2)
        out[b, half * TQ : (half + 1) * TQ, :] = res.results[c]["outT"].T
    return out


# revision 9
# speedup vs baseline: 14087.1120x; 14087.1120x over previous
"""Trainium2 Bass kernel for nn_DecoderLayer (self-attn + cross-attn + FFN layer).

Sharding: 8 cores = (batch, query-half). Core c handles batch c//2 and query rows
[512*(c%2), 512*(c%2)+512). Each core computes the full layer for its 512 query
tokens; K/V work over the full 1024 key tokens is duplicated between the two
cores of a batch (cheaper than collectives at this size).

On-chip layout: transposed activations (features on partitions, tokens free), so
projections chain with no on-chip transposes. Attention uses the transposed-
scores formulation: scores^T[k,q] = K^T-chunk (stationary) @ Q^T (moving), plus
an identity-matmul accumulating the host-premultiplied (-1e9) mask into the same
PSUM; exp runs on the scalar engine straight out of PSUM; the AV matmul consumes
probs^T directly with V stored tokens-on-partitions, and a fused ones column in
the V tile yields the softmax denominators in the same matmul. All matmuls run
as float32r (full-rate fp32, ~2e-4 rel err). LayerNorm reduces across partitions
via ones-vector matmuls; per-token scale/bias broadcast via K=1 ones matmuls.

Host side: pre-transposes per-core activations (rotating so each core's own
query block sits in columns [0:512) — the key axis is permutation-invariant as
long as the mask rows are permuted identically), pre-packs weight tiles, and
transposes the returned out^T back.
"""

import numpy as np

B, T, D, H, DH, FFN = 4, 1024, 1024, 16, 64, 4096
EPS = 1e-5
P = 128
ND = D // P          # 8 contraction chunks over D
NF = FFN // P        # 32 ffn chunks
TQ = 512             # query tokens per core
NKC = T // P         # 8 key chunks
NHP = H // 2         # 8 head pairs

_CACHE = {}


def _split_waits(nc, maxw=1):
    """Walrus in this toolchain encodes at most one sem-wait per TPB
    instruction; distribute excess waits onto preceding same-engine NOPs."""
    import bass_rust
    import concourse.mybir as mybir

    for bbw in nc.main_func.blocks:
        insts = bbw.instructions
        out = []
        changed = False
        for inst in insts:
            si = inst.sync_info
            waits = list(si.on_wait or []) if si is not None else []
            if len(waits) > maxw:
                changed = True
                extra = waits[: len(waits) - maxw]
                si.on_wait = waits[len(waits) - maxw :]
                for i in range(0, len(extra), maxw):
                    nop = mybir.InstNoOp(
                        name=nc.get_next_instruction_name(),
                        ins=[],
                        outs=[],
                        engine=inst.engine,
                        sync_info=bass_rust.SyncInfo(
                            on_wait=extra[i : i + maxw], on_update=[]
                        ),
                    )
                    nc.register_instruction(nop, overwrite=True)
                    out.append(nop)
            out.append(inst)
        if changed:
            bbw.instructions[:] = out


def _build_decoder():
    import concourse.bass as bass
    import concourse.mybir as mybir
    import concourse.tile as tile
    from contextlib import ExitStack

    F32 = mybir.dt.float32
    F32R = mybir.dt.float32r
    AF = mybir.ActivationFunctionType
    ALU = mybir.AluOpType

    nc = bass.Bass()

    def din(name, shape):
        return nc.dram_tensor(name, shape, F32, kind="ExternalInput")

    tT = din("tT", [D, T])        # target^T, own query block first
    sT = din("sT", [D, T])        # source^T
    mT = din("mT", [T, TQ])       # mask bias^T (-1e9 where masked), rows permuted
    hident = din("hident", [P, P])
    hones = din("hones", [P, P])
    wq1 = din("wq1", [ND, ND, P, P]);  bq1 = din("bq1", [D])
    wk1 = din("wk1", [ND, ND, P, P]);  bk1 = din("bk1", [D])
    wv1 = din("wv1", [D, D]);          bv1 = din("bv1", [1, D])
    wo1 = din("wo1", [ND, ND, P, P]);  bo1 = din("bo1", [D])
    wq2 = din("wq2", [ND, ND, P, P]);  bq2 = din("bq2", [D])
    wk2 = din("wk2", [ND, ND, P, P]);  bk2 = din("bk2", [D])
    wv2 = din("wv2", [D, D]);          bv2 = din("bv2", [1, D])
    wo2 = din("wo2", [ND, ND, P, P]);  bo2 = din("bo2", [D])
    w1r = din("w1r", [NF, ND, P, P]);  b1 = din("b1", [FFN])
    w2r = din("w2r", [ND, NF, P, P]);  b2 = din("b2", [D])
    ln_g = din("ln_g", [3, D])
    ln_b = din("ln_b", [3, D])
    outT = nc.dram_tensor("outT", [D, TQ], F32, kind="ExternalOutput")

    with tile.TileContext(nc) as tc, ExitStack() as glob:
        consts = glob.enter_context(tc.tile_pool(name="consts", bufs=1))
        smallp = glob.enter_context(tc.tile_pool(name="smalls", bufs=1))
        ctxp = glob.enter_context(tc.tile_pool(name="ctx", bufs=8))
        residp = glob.enter_context(tc.tile_pool(name="resid", bufs=8))

        ident = consts.tile([P, P], F32R)
        nc.sync.dma_start(ident[:], hident[:].bitcast(F32R))
        ones_c = consts.tile([1, P], F32R)
        nc.sync.dma_start(ones_c[:], hones[0:1, :].bitcast(F32R))
        ones_r = consts.tile([P, 1], F32R)
        nc.sync.dma_start(ones_r[:], hones[:, 0:1].bitcast(F32R))
        ones16 = consts.tile([P, 16], F32)
        nc.sync.dma_start(ones16[:], hones[:, 0:16])
        eps_t = consts.tile([1, 1], F32)
        nc.vector.memset(eps_t, EPS)
        lng = consts.tile([P, 3, ND], F32)
        nc.sync.dma_start(lng[:], ln_g.rearrange("l (c p) -> p l c", p=P))
        lnb = consts.tile([P, 3, ND], F32)
        nc.sync.dma_start(lnb[:], ln_b.rearrange("l (c p) -> p l c", p=P))

        def f32(ap):
            return ap.bitcast(F32)

        def load_rows(pool, dram, ncols, tag):
            tiles = []
            for c in range(dram.shape[0] // P):
                t_ = pool.tile([P, ncols], F32R, tag=tag)
                nc.sync.dma_start(t_[:], dram[c * P : (c + 1) * P, :].bitcast(F32R))
                tiles.append(t_)
            return tiles

        def proj_chunk(wr, bvec, jc, x_tiles, cols, out_pool, out_tag, wpool,
                       pspool, pstag="pps", func=AF.Identity, out_dtype=F32R):
            """One output-feature chunk jc of out^T = func(W.T @ X^T + b)."""
            ndc = wr.shape[1]
            wsl = wpool.tile([P, ndc, P], F32R, tag="wsl")
            nc.sync.dma_start(wsl[:],
                              wr[jc].rearrange("c p m -> p c m").bitcast(F32R))
            bt = wpool.tile([P, 1], F32, tag="bt")
            nc.sync.dma_start(bt[:], bvec[jc * P : (jc + 1) * P, None])
            outs = []
            for c0, cn in cols:
                ps = pspool.tile([P, 512], F32, tag=pstag)
                for dc in range(ndc):
                    nc.tensor.matmul(ps[:], wsl[:, dc, :],
                                     x_tiles[dc][:, c0 : c0 + cn],
                                     start=(dc == 0), stop=(dc == ndc - 1))
                o = out_pool.tile([P, cn], out_dtype, tag=out_tag)
                nc.scalar.activation(out=o[:], in_=ps[:, 0:cn], func=func,
                                     bias=bt[:], scale=1.0)
                outs.append(o)
            return outs

        def proj_tokens(wv, bv, x_tiles, vpool, vwp, pspool):
            """V with fused ones column, tokens on partitions:
            vext[sc] [P, H, DH+1]."""
            vtiles = []
            for sc in range(NKC):
                vt = vpool.tile([P, H, DH + 1], F32R, tag="vext")
                nc.vector.tensor_copy(out=vt[:, :, DH : DH + 1],
                                      in_=ones16[:, :, None])
                vtiles.append(vt)
            for q in range(4):  # quarter = 256 features = 4 heads
                wslab = vwp.tile([P, ND, 256], F32R, tag="vwsl")
                nc.sync.dma_start(
                    wslab[:],
                    wv[:, q * 256 : (q + 1) * 256].rearrange(
                        "(c p) n -> p c n", p=P).bitcast(F32R))
                bvt = vwp.tile([1, 256], F32R, tag="vbias")
                nc.sync.dma_start(bvt[:],
                                  bv[:, q * 256 : (q + 1) * 256].bitcast(F32R))
                for sc in range(NKC):
                    ps = pspool.tile([P, 256], F32, tag="vps")
                    for dc in range(ND):
                        nc.tensor.matmul(ps[:],
                                         x_tiles[dc][:, sc * P : (sc + 1) * P],
                                         wslab[:, dc, :],
                                         start=(dc == 0), stop=False)
                    nc.tensor.matmul(ps[:], ones_c[:], bvt[:],
                                     start=False, stop=True)
                    dst = vtiles[sc][:, q * 4 : (q + 1) * 4, 0:DH]
                    nc.scalar.activation(
                        out=dst, in_=ps[:].rearrange("p (h d) -> p h d", h=4),
                        func=AF.Copy)
            return vtiles

        def attention(qt_tiles, kt_producer, v_tiles, mask_tiles, pspools,
                      probsp):
            """qt_tiles: 8 [P, TQ] (2 heads per tile); kt_producer(hp) -> 2
            half tiles [P, 512]; v_tiles: 8 [P, H, DH+1]; mask_tiles or None."""
            scp, ctxps, bcps = pspools
            ctx_tiles = []
            for hp in range(NHP):
                kt_h = kt_producer(hp)
                ctx_t = ctxp.tile([P, TQ], F32R, tag="ctxT")
                ctx_tiles.append(ctx_t)
                qa = qt_tiles[hp][0:64, :]
                qb = qt_tiles[hp][64:128, :]
                psc_a = ctxps.tile([65, TQ], F32, tag="ctxps_a")
                psc_b = ctxps.tile([65, TQ], F32, tag="ctxps_b")
                pending = None
                for kc in range(NKC):
                    kt_t = kt_h[kc // 4]
                    kcol = (kc % 4) * P
                    psA = scp.tile([P, TQ], F32, tag="scA")
                    psB = scp.tile([P, TQ], F32, tag="scB")
                    ka = kt_t[0:64, kcol : kcol + P]
                    kb = kt_t[64:128, kcol : kcol + P]
                    has_mask = mask_tiles is not None
                    nc.tensor.matmul(psA[:], ka, qa, start=True,
                                     stop=not has_mask, tile_position=(0, 0))
                    nc.tensor.matmul(psB[:], kb, qb, start=True,
                                     stop=not has_mask, tile_position=(64, 0))
                    if has_mask:
                        m = mask_tiles[kc]
                        nc.tensor.matmul(psA[:], ident[:], m[:],
                                         start=False, stop=True)
                        nc.tensor.matmul(psB[:], ident[:], m[:],
                                         start=False, stop=True)
                    pA = probsp.tile([P, TQ], F32R, tag="prA")
                    pB = probsp.tile([P, TQ], F32R, tag="prB")
                    nc.scalar.activation(out=pA[:], in_=psA[:], func=AF.Exp,
                                         scale=0.125)
                    nc.scalar.activation(out=pB[:], in_=psB[:], func=AF.Exp,
                                         scale=0.125)
                    if pending is not None:
                        kp, ppA, ppB = pending
                        nc.tensor.matmul(psc_a[:], v_tiles[kp][:, 2 * hp, :],
                                         ppA[:], start=(kp == 0), stop=False)
                        nc.tensor.matmul(psc_b[:], v_tiles[kp][:, 2 * hp + 1, :],
                                         ppB[:], start=(kp == 0), stop=False)
                    pending = (kc, pA, pB)
                kp, ppA, ppB = pending
                nc.tensor.matmul(psc_a[:], v_tiles[kp][:, 2 * hp, :], ppA[:],
                                 start=False, stop=True)
                nc.tensor.matmul(psc_b[:], v_tiles[kp][:, 2 * hp + 1, :], ppB[:],
                                 start=False, stop=True)
                for par, psc in ((0, psc_a), (1, psc_b)):
                    rf = smallp.tile([1, TQ], F32, tag="recf")
                    nc.vector.reciprocal(out=rf[:], in_=psc[64:65, :])
                    rr = smallp.tile([1, TQ], F32R, tag="recr")
                    nc.vector.tensor_copy(out=rr[:], in_=rf[:])
                    psb = bcps.tile([64, TQ], F32, tag="bc")
                    nc.tensor.matmul(psb[:], ones_c[:, 0:64], rr[:],
                                     start=True, stop=True)
                    csb = probsp.tile([64, TQ], F32, tag="csb")
                    nc.scalar.activation(out=csb[:], in_=psc[0:64, :],
                                         func=AF.Copy)
                    nc.vector.tensor_tensor(
                        out=ctx_t[par * 64 : (par + 1) * 64, :], in0=csb[:],
                        in1=psb[:], op=ALU.mult)
            return ctx_tiles

        def layer_norm(pre_tiles, ln_idx, out_pool, out_tag, pools,
                       out_dtype=F32R):
            """LN across the partition (feature) axis of 8 F32R [P, TQ] tiles."""
            tmpp, statps, bcps = pools
            sq_tiles = []
            for dc in range(ND):
                sq = tmpp.tile([P, TQ], F32R, tag="lnsq")
                nc.vector.tensor_tensor(out=sq[:], in0=f32(pre_tiles[dc][:]),
                                        in1=f32(pre_tiles[dc][:]), op=ALU.mult)
                sq_tiles.append(sq)
            ps1 = statps.tile([1, TQ], F32, tag="lns1")
            ps2 = statps.tile([1, TQ], F32, tag="lns2")
            for dc in range(ND):
                nc.tensor.matmul(ps1[:], ones_r[:], pre_tiles[dc][:],
                                 start=(dc == 0), stop=(dc == ND - 1))
            for dc in range(ND):
                nc.tensor.matmul(ps2[:], ones_r[:], sq_tiles[dc][:],
                                 start=(dc == 0), stop=(dc == ND - 1))
            mu = smallp.tile([1, TQ], F32, tag="lnmu")
            nc.vector.tensor_scalar_mul(out=mu[:], in0=ps1[:], scalar1=1.0 / D)
            ex2 = smallp.tile([1, TQ], F32, tag="lnex2")
            nc.vector.tensor_scalar_mul(out=ex2[:], in0=ps2[:], scalar1=1.0 / D)
            var = smallp.tile([1, TQ], F32, tag="lnvar")
            nc.vector.tensor_tensor(out=var[:], in0=mu[:], in1=mu[:], op=ALU.mult)
            nc.vector.tensor_tensor(out=var[:], in0=ex2[:], in1=var[:],
                                    op=ALU.subtract)
            sd = smallp.tile([1, TQ], F32, tag="lnsd")
            nc.scalar.activation(out=sd[:], in_=var[:], func=AF.Sqrt,
                                 bias=eps_t[:], scale=1.0)
            rsf = smallp.tile([1, TQ], F32, tag="lnrsf")
            nc.vector.reciprocal(out=rsf[:], in_=sd[:])
            cmul = smallp.tile([1, TQ], F32, tag="lncm")
            nc.vector.tensor_tensor(out=cmul[:], in0=mu[:], in1=rsf[:],
                                    op=ALU.mult)
            cneg = smallp.tile([1, TQ], F32R, tag="lncn")
            nc.vector.tensor_scalar_mul(out=cneg[:], in0=cmul[:], scalar1=-1.0)
            rsig = smallp.tile([1, TQ], F32R, tag="lnrs")
            nc.vector.tensor_copy(out=rsig[:], in_=rsf[:])
            psa = bcps.tile([P, TQ], F32, tag="lnbca")
            nc.tensor.matmul(psa[:], ones_c[:], rsig[:], start=True, stop=True)
            psc = bcps.tile([P, TQ], F32, tag="lnbcc")
            nc.tensor.matmul(psc[:], ones_c[:], cneg[:], start=True, stop=True)
            out_tiles = []
            for dc in range(ND):
                t1 = tmpp.tile([P, TQ], F32, tag="lnt1")
                nc.vector.tensor_tensor(out=t1[:], in0=f32(pre_tiles[dc][:]),
                                        in1=psa[:], op=ALU.mult)
                t2 = tmpp.tile([P, TQ], F32, tag="lnt2")
                nc.vector.tensor_tensor(out=t2[:], in0=t1[:], in1=psc[:],
                                        op=ALU.add)
                o = out_pool.tile([P, TQ], out_dtype, tag=out_tag)
                nc.vector.tensor_scalar(
                    out=o[:], in0=t2[:],
                    scalar1=lng[:, ln_idx, dc : dc + 1],
                    scalar2=lnb[:, ln_idx, dc : dc + 1],
                    op0=ALU.mult, op1=ALU.add)
                out_tiles.append(o)
            return out_tiles

        def oproj_resid_ln(wo, bo, ctx_tiles, resid_f32_aps, ln_idx, out_tag,
                           stack):
            wpool = stack.enter_context(tc.tile_pool(name="owp", bufs=3))
            pps = stack.enter_context(tc.tile_pool(name="ops", bufs=2,
                                                   space="PSUM"))
            tmpp = stack.enter_context(tc.tile_pool(name="otmp", bufs=3))
            statps = stack.enter_context(
                tc.tile_pool(name="olnst", bufs=1, space="PSUM"))
            bcps = stack.enter_context(
                tc.tile_pool(name="olnbc", bufs=1, space="PSUM"))
            prep = stack.enter_context(tc.tile_pool(name="opre", bufs=8))
            pre_tiles = []
            for jc in range(ND):
                (osb,) = proj_chunk(wo, bo, jc, ctx_tiles, [(0, TQ)], tmpp,
                                    "osb", wpool, pps, out_dtype=F32)
                pre = prep.tile([P, TQ], F32R, tag="pre")
                nc.vector.tensor_tensor(out=pre[:], in0=osb[:],
                                        in1=resid_f32_aps[jc], op=ALU.add)
                pre_tiles.append(pre)
            return layer_norm(pre_tiles, ln_idx, residp, out_tag,
                              (tmpp, statps, bcps))

        # ================= phases =================
        with ExitStack() as ph_t:
            ttp = ph_t.enter_context(tc.tile_pool(name="tTp", bufs=8))
            tT_tiles = load_rows(ttp, tT, T, "tT")

            with ExitStack() as ph_sa:
                qtp = ph_sa.enter_context(tc.tile_pool(name="qtp", bufs=8))
                vxp = ph_sa.enter_context(tc.tile_pool(name="vxp", bufs=8))
                wp = ph_sa.enter_context(tc.tile_pool(name="saw", bufs=3))
                with ExitStack() as ph_p1:
                    pp = ph_p1.enter_context(
                        tc.tile_pool(name="p1ps", bufs=4, space="PSUM"))
                    qt_tiles = []
                    for jc in range(ND):
                        (q_,) = proj_chunk(wq1, bq1, jc, tT_tiles, [(0, TQ)],
                                           qtp, "qt", wp, pp)
                        qt_tiles.append(q_)
                    vwp = ph_p1.enter_context(tc.tile_pool(name="vwp", bufs=2))
                    v_tiles = proj_tokens(wv1, bv1, tT_tiles, vxp, vwp, pp)

                with ExitStack() as ph_a1:
                    mtp = ph_a1.enter_context(tc.tile_pool(name="mtp", bufs=8))
                    m_tiles = load_rows(mtp, mT, TQ, "mt")
                    ktp = ph_a1.enter_context(tc.tile_pool(name="ktp", bufs=3))
                    ktps = ph_a1.enter_context(
                        tc.tile_pool(name="ktps", bufs=1, space="PSUM"))
                    scp = ph_a1.enter_context(
                        tc.tile_pool(name="scps", bufs=2, space="PSUM"))
                    ctxps = ph_a1.enter_context(
                        tc.tile_pool(name="ctxps", bufs=1, space="PSUM"))
                    bcps = ph_a1.enter_context(
                        tc.tile_pool(name="bcps", bufs=1, space="PSUM"))

                    def kt1_producer(hp):
                        return proj_chunk(wk1, bk1, hp, tT_tiles,
                                          [(0, 512), (512, 512)], ktp, "kt",
                                          wp, ktps, pstag="ktps")

                    probsp1 = ph_a1.enter_context(
                        tc.tile_pool(name="probs1", bufs=3))
                    ctx1 = attention(qt_tiles, kt1_producer, v_tiles, m_tiles,
                                     (scp, ctxps, bcps), probsp1)

            with ExitStack() as ph_o1:
                x_tiles = oproj_resid_ln(
                    wo1, bo1, ctx1, [f32(t_[:, 0:TQ]) for t_ in tT_tiles], 0,
                    "resid", ph_o1)

        with ExitStack() as ph_s:
            stp = ph_s.enter_context(tc.tile_pool(name="sTp", bufs=8))
            sT_tiles = load_rows(stp, sT, T, "sT")

            with ExitStack() as ph_ca:
                qtp2 = ph_ca.enter_context(tc.tile_pool(name="qtp2", bufs=8))
                vxp2 = ph_ca.enter_context(tc.tile_pool(name="vxp2", bufs=8))
                wp2 = ph_ca.enter_context(tc.tile_pool(name="caw", bufs=3))
                with ExitStack() as ph_p2:
                    pp = ph_p2.enter_context(
                        tc.tile_pool(name="p2ps", bufs=4, space="PSUM"))
                    qt2 = []
                    for jc in range(ND):
                        (q_,) = proj_chunk(wq2, bq2, jc, x_tiles, [(0, TQ)],
                                           qtp2, "qt2", wp2, pp)
                        qt2.append(q_)
                    vwp2 = ph_p2.enter_context(tc.tile_pool(name="vwp2", bufs=2))
                    v2 = proj_tokens(wv2, bv2, sT_tiles, vxp2, vwp2, pp)

                with ExitStack() as ph_a2:
                    ktp2 = ph_a2.enter_context(tc.tile_pool(name="ktp2", bufs=3))
                    ktps2 = ph_a2.enter_context(
                        tc.tile_pool(name="ktps2", bufs=1, space="PSUM"))
                    scp = ph_a2.enter_context(
                        tc.tile_pool(name="scps2", bufs=2, space="PSUM"))
                    ctxps = ph_a2.enter_context(
                        tc.tile_pool(name="ctxps2", bufs=1, space="PSUM"))
                    bcps = ph_a2.enter_context(
                        tc.tile_pool(name="bcps2", bufs=1, space="PSUM"))

                    def kt2_producer(hp):
                        return proj_chunk(wk2, bk2, hp, sT_tiles,
                                          [(0, 512), (512, 512)], ktp2, "kt2",
                                          wp2, ktps2, pstag="ktps2")

                    probsp2 = ph_a2.enter_context(
                        tc.tile_pool(name="probs2", bufs=3))
                    ctx2 = attention(qt2, kt2_producer, v2, None,
                                     (scp, ctxps, bcps), probsp2)

            with ExitStack() as ph_o2:
                y_tiles = oproj_resid_ln(
                    wo2, bo2, ctx2, [f32(t_[:]) for t_ in x_tiles], 1, "resid",
                    ph_o2)

        # ---- FFN + LN3 ----
        with ExitStack() as ph_f:
            hpool = ph_f.enter_context(tc.tile_pool(name="hp", bufs=12))
            faccp = ph_f.enter_context(tc.tile_pool(name="facc", bufs=8))
            w1p = ph_f.enter_context(tc.tile_pool(name="w1p", bufs=3))
            w2p = ph_f.enter_context(tc.tile_pool(name="w2p", bufs=3))
            b2p = ph_f.enter_context(tc.tile_pool(name="b2p", bufs=8))
            ps1p = ph_f.enter_context(
                tc.tile_pool(name="fps1", bufs=2, space="PSUM"))
            ps2p = ph_f.enter_context(
                tc.tile_pool(name="fps2", bufs=2, space="PSUM"))
            facc = [None] * ND
            b2t = []
            for jc in range(ND):
                bt = b2p.tile([P, 1], F32, tag="b2t")
                nc.sync.dma_start(bt[:], b2[jc * P : (jc + 1) * P, None])
                b2t.append(bt)
            for g in range(4):
                h_tiles = []
                for kl in range(8):
                    k = g * 8 + kl
                    wsl = w1p.tile([P, ND, P], F32R, tag="w1sl")
                    nc.sync.dma_start(
                        wsl[:], w1r[k].rearrange("c p m -> p c m"))
                    bt = w1p.tile([P, 1], F32, tag="b1t")
                    nc.sync.dma_start(bt[:], b1[k * P : (k + 1) * P, None])
                    ps = ps1p.tile([P, TQ], F32, tag="w1ps")
                    for dc in range(ND):
                        nc.tensor.matmul(ps[:], wsl[:, dc, :], y_tiles[dc][:],
                                         start=(dc == 0), stop=(dc == ND - 1))
                    ht = hpool.tile([P, TQ], F32R, tag="h")
                    nc.scalar.activation(out=ht[:], in_=ps[:], func=AF.Relu,
                                         bias=bt[:], scale=1.0)
                    h_tiles.append(ht)
                for jc in range(ND):
                    wsl2 = w2p.tile([P, 8, P], F32R, tag="w2sl")
                    nc.sync.dma_start(
                        wsl2[:],
                        w2r[jc, g * 8 : (g + 1) * 8].rearrange(
                            "c p m -> p c m").bitcast(F32R))
                    ps = ps2p.tile([P, TQ], F32, tag="w2ps")
                    for dcl in range(8):
                        nc.tensor.matmul(ps[:], wsl2[:, dcl, :],
                                         h_tiles[dcl][:],
                                         start=(dcl == 0), stop=(dcl == 7))
                    if g == 0:
                        fa = faccp.tile([P, TQ], F32, tag="fa")
                        nc.scalar.activation(out=fa[:], in_=ps[:],
                                             func=AF.Identity,
                                             bias=b2t[jc][:], scale=1.0)
                        facc[jc] = fa
                    else:
                        nc.vector.tensor_tensor(out=facc[jc][:],
                                                in0=facc[jc][:], in1=ps[:],
                                                op=ALU.add)
            with ExitStack() as ph_l3:
                tmpp = ph_l3.enter_context(tc.tile_pool(name="l3tmp", bufs=3))
                statps = ph_l3.enter_context(
                    tc.tile_pool(name="l3st", bufs=1, space="PSUM"))
                bcps2 = ph_l3.enter_context(
                    tc.tile_pool(name="l3bc", bufs=1, space="PSUM"))
                prep = ph_l3.enter_context(tc.tile_pool(name="l3pre", bufs=8))
                outp = ph_l3.enter_context(tc.tile_pool(name="outp", bufs=3))
                pre3 = []
                for jc in range(ND):
                    pre = prep.tile([P, TQ], F32R, tag="pre3")
                    nc.vector.tensor_tensor(out=pre[:], in0=facc[jc][:],
                                            in1=f32(y_tiles[jc][:]),
                                            op=ALU.add)
                    pre3.append(pre)
                out_tiles = layer_norm(pre3, 2, outp, "out",
                                       (tmpp, statps, bcps2), out_dtype=F32)
                for jc in range(ND):
                    nc.sync.dma_start(outT[jc * P : (jc + 1) * P, :],
                                      out_tiles[jc][:])

    _split_waits(nc)
    return nc


def _host_pack(params):
    def pk(w):
        w = np.ascontiguousarray(np.asarray(w, dtype=np.float32))
        ndc = w.shape[0] // P
        njc = w.shape[1] // P
        return np.ascontiguousarray(
            w.reshape(ndc, P, njc, P).transpose(2, 0, 1, 3))

    p = params
    return {
        "wq1": pk(p["sa_q_W"]), "bq1": np.asarray(p["sa_q_b"], np.float32),
        "wk1": pk(p["sa_k_W"]), "bk1": np.asarray(p["sa_k_b"], np.float32),
        "wv1": np.ascontiguousarray(np.asarray(p["sa_v_W"], np.float32)),
        "bv1": np.asarray(p["sa_v_b"], np.float32).reshape(1, D),
        "wo1": pk(p["sa_o_W"]), "bo1": np.asarray(p["sa_o_b"], np.float32),
        "wq2": pk(p["ca_q_W"]), "bq2": np.asarray(p["ca_q_b"], np.float32),
        "wk2": pk(p["ca_k_W"]), "bk2": np.asarray(p["ca_k_b"], np.float32),
        "wv2": np.ascontiguousarray(np.asarray(p["ca_v_W"], np.float32)),
        "bv2": np.asarray(p["ca_v_b"], np.float32).reshape(1, D),
        "wo2": pk(p["ca_o_W"]), "bo2": np.asarray(p["ca_o_b"], np.float32),
        "w1r": pk(p["ffn_W1"]), "b1": np.asarray(p["ffn_b1"], np.float32),
        "w2r": pk(p["ffn_W2"]), "b2": np.asarray(p["ffn_b2"], np.float32),
        "ln_g": np.stack([np.asarray(p["ln1_g"], np.float32),
                          np.asarray(p["ln2_g"], np.float32),
                          np.asarray(p["ln3_g"], np.float32)]),
        "ln_b": np.stack([np.asarray(p["ln1_b"], np.float32),
                          np.asarray(p["ln2_b"], np.float32),
                          np.asarray(p["ln3_b"], np.float32)]),
        "hident": np.eye(P, dtype=np.float32),
        "hones": np.ones((P, P), dtype=np.float32),
    }


def kernel(target, source, pad_masked, params):
    from concourse.bass_utils import run_bass_kernel_spmd

    target = np.asarray(target, dtype=np.float32)
    source = np.asarray(source, dtype=np.float32)
    mask = np.asarray(pad_masked)

    wshared = _host_pack(params)

    in_maps = []
    for c in range(8):
        b, half = divmod(c, 2)
        q0 = half * TQ
        perm = np.concatenate([np.arange(q0, q0 + TQ),
                               np.arange(0, q0),
                               np.arange(q0 + TQ, T)]).astype(np.int64)
        m = dict(wshared)
        tTb = target[b].T  # [D, T]
        m["tT"] = np.ascontiguousarray(tTb[:, perm])
        m["sT"] = np.ascontiguousarray(source[b].T)
        mrows = (mask[b, q0 : q0 + TQ, :].astype(np.float32)
                 * np.float32(-1e9)).T  # [T keys, TQ]
        m["mT"] = np.ascontiguousarray(mrows[perm, :])
        in_maps.append(m)

    if "nc" not in _CACHE:
        _CACHE["nc"] = _build_decoder()
    res = run_bass_kernel_spmd(_CACHE["nc"], in_maps, core_ids=list(range(8)))

    out = np.empty((B, T, D), dtype=np.float32)
    for c in range(8):
        b, half = divmod(c, 2)
        out[b, half * TQ : (half + 1) * TQ, :] = res.results[c]["outT"].T
    return out


# revision 12
# speedup vs baseline: 14369.8301x; 1.0201x over previous
"""Trainium2 Bass kernel for nn_DecoderLayer (self-attn + cross-attn + FFN layer).

Sharding: 8 cores = (batch, query-half). Core c handles batch c//2 and query rows
[512*(c%2), 512*(c%2)+512). Each core computes the full layer for its 512 query
tokens; K/V work over the full 1024 key tokens is duplicated between the two
cores of a batch (cheaper than collectives at this size).

On-chip layout: transposed activations (features on partitions, tokens free), so
projections chain with no on-chip transposes. Attention uses the transposed-
scores formulation: scores^T[k,q] = K^T-chunk (stationary) @ Q^T (moving), plus
an identity-matmul accumulating the host-premultiplied (-1e9) mask into the same
PSUM; exp runs on the scalar engine straight out of PSUM; the AV matmul consumes
probs^T directly with V stored tokens-on-partitions, and a fused ones column in
the V tile yields the softmax denominators in the same matmul. All matmuls run
as float32r (full-rate fp32, ~2e-4 rel err). LayerNorm reduces across partitions
via ones-vector matmuls; per-token scale/bias broadcast via K=1 ones matmuls.

Host side: pre-transposes per-core activations (rotating so each core's own
query block sits in columns [0:512) — the key axis is permutation-invariant as
long as the mask rows are permuted identically), pre-packs weight tiles, and
transposes the returned out^T back.
"""

import numpy as np

B, T, D, H, DH, FFN = 4, 1024, 1024, 16, 64, 4096
EPS = 1e-5
P = 128
ND = D // P          # 8 contraction chunks over D
NF = FFN // P        # 32 ffn chunks
TQ = 512             # query tokens per core
NKC = T // P         # 8 key chunks
NHP = H // 2         # 8 head pairs

_CACHE = {}


def _split_waits(nc, maxw=1):
    """Walrus in this toolchain encodes at most one sem-wait per TPB
    instruction; distribute excess waits onto preceding same-engine NOPs."""
    import bass_rust
    import concourse.mybir as mybir

    for bbw in nc.main_func.blocks:
        insts = bbw.instructions
        out = []
        changed = False
        for inst in insts:
            si = inst.sync_info
            waits = list(si.on_wait or []) if si is not None else []
            if len(waits) > maxw:
                changed = True
                extra = waits[: len(waits) - maxw]
                si.on_wait = waits[len(waits) - maxw :]
                for i in range(0, len(extra), maxw):
                    nop = mybir.InstNoOp(
                        name=nc.get_next_instruction_name(),
                        ins=[],
                        outs=[],
                        engine=inst.engine,
                        sync_info=bass_rust.SyncInfo(
                            on_wait=extra[i : i + maxw], on_update=[]
                        ),
                    )
                    nc.register_instruction(nop, overwrite=True)
                    out.append(nop)
            out.append(inst)
        if changed:
            bbw.instructions[:] = out


def _build_decoder():
    import concourse.bass as bass
    import concourse.mybir as mybir
    import concourse.tile as tile
    from contextlib import ExitStack

    F32 = mybir.dt.float32
    F32R = mybir.dt.float32r
    AF = mybir.ActivationFunctionType
    ALU = mybir.AluOpType

    nc = bass.Bass()

    def din(name, shape):
        return nc.dram_tensor(name, shape, F32, kind="ExternalInput")

    tT = din("tT", [D, T])        # target^T, own query block first
    sT = din("sT", [D, T])        # source^T
    mT = din("mT", [T, TQ])       # mask bias^T (-1e9 where masked), rows permuted
    hident = din("hident", [P, P])
    hones = din("hones", [P, P])
    wq1 = din("wq1", [ND, ND, P, P]);  bq1 = din("bq1", [D])
    wk1 = din("wk1", [ND, ND, P, P]);  bk1 = din("bk1", [D])
    wv1 = din("wv1", [D, D]);          bv1 = din("bv1", [1, D])
    wo1 = din("wo1", [ND, ND, P, P]);  bo1 = din("bo1", [D])
    wq2 = din("wq2", [ND, ND, P, P]);  bq2 = din("bq2", [D])
    wk2 = din("wk2", [ND, ND, P, P]);  bk2 = din("bk2", [D])
    wv2 = din("wv2", [D, D]);          bv2 = din("bv2", [1, D])
    wo2 = din("wo2", [ND, ND, P, P]);  bo2 = din("bo2", [D])
    w1r = din("w1r", [NF, ND, P, P]);  b1 = din("b1", [FFN])
    w2r = din("w2r", [ND, NF, P, P]);  b2 = din("b2", [D])
    ln_g = din("ln_g", [3, D])
    ln_b = din("ln_b", [3, D])
    outT = nc.dram_tensor("outT", [D, TQ], F32, kind="ExternalOutput")

    with tile.TileContext(nc) as tc, ExitStack() as glob:
        consts = glob.enter_context(tc.tile_pool(name="consts", bufs=1))
        smallp = glob.enter_context(tc.tile_pool(name="smalls", bufs=1))
        ctxp = glob.enter_context(tc.tile_pool(name="ctx", bufs=8))
        w1p = glob.enter_context(tc.tile_pool(name="w1p", bufs=4))
        residp = glob.enter_context(tc.tile_pool(name="resid", bufs=8))

        ident = consts.tile([P, P], F32R)
        nc.sync.dma_start(ident[:], hident[:].bitcast(F32R))
        ones_c = consts.tile([1, P], F32R)
        nc.sync.dma_start(ones_c[:], hones[0:1, :].bitcast(F32R))
        ones_r = consts.tile([P, 1], F32R)
        nc.sync.dma_start(ones_r[:], hones[:, 0:1].bitcast(F32R))
        ones16 = consts.tile([P, 16], F32)
        nc.sync.dma_start(ones16[:], hones[:, 0:16])
        eps_t = consts.tile([1, 1], F32)
        nc.vector.memset(eps_t, EPS)
        lng = consts.tile([P, 3, ND], F32)
        nc.sync.dma_start(lng[:], ln_g.rearrange("l (c p) -> p l c", p=P))
        lnb = consts.tile([P, 3, ND], F32)
        nc.sync.dma_start(lnb[:], ln_b.rearrange("l (c p) -> p l c", p=P))

        def f32(ap):
            return ap.bitcast(F32)

        def load_rows(pool, dram, ncols, tag):
            tiles = []
            for c in range(dram.shape[0] // P):
                t_ = pool.tile([P, ncols], F32R, tag=tag)
                nc.sync.dma_start(t_[:], dram[c * P : (c + 1) * P, :].bitcast(F32R))
                tiles.append(t_)
            return tiles

        def proj_chunk(wr, bvec, jc, x_tiles, cols, out_pool, out_tag, wpool,
                       pspool, pstag="pps", func=AF.Identity, out_dtype=F32R):
            """One output-feature chunk jc of out^T = func(W.T @ X^T + b)."""
            ndc = wr.shape[1]
            wsl = wpool.tile([P, ndc, P], F32R, tag="wsl")
            nc.sync.dma_start(wsl[:],
                              wr[jc].rearrange("c p m -> p c m").bitcast(F32R))
            bt = wpool.tile([P, 1], F32, tag="bt")
            nc.sync.dma_start(bt[:], bvec[jc * P : (jc + 1) * P, None])
            outs = []
            for c0, cn in cols:
                ps = pspool.tile([P, 512], F32, tag=pstag)
                for dc in range(ndc):
                    nc.tensor.matmul(ps[:], wsl[:, dc, :],
                                     x_tiles[dc][:, c0 : c0 + cn],
                                     start=(dc == 0), stop=(dc == ndc - 1))
                o = out_pool.tile([P, cn], out_dtype, tag=out_tag)
                nc.scalar.activation(out=o[:], in_=ps[:, 0:cn], func=func,
                                     bias=bt[:], scale=1.0)
                outs.append(o)
            return outs

        def proj_tokens(wv, bv, x_tiles, vpool, vwp, pspool):
            """V with fused ones column, tokens on partitions:
            vext[sc] [P, H, DH+1]."""
            vtiles = []
            for sc in range(NKC):
                vt = vpool.tile([P, H, DH + 1], F32R, tag="vext")
                nc.vector.tensor_copy(out=vt[:, :, DH : DH + 1],
                                      in_=ones16[:, :, None])
                vtiles.append(vt)
            for q in range(4):  # quarter = 256 features = 4 heads
                wslab = vwp.tile([P, ND, 256], F32R, tag="vwsl")
                nc.sync.dma_start(
                    wslab[:],
                    wv[:, q * 256 : (q + 1) * 256].rearrange(
                        "(c p) n -> p c n", p=P).bitcast(F32R))
                bvt = vwp.tile([1, 256], F32R, tag="vbias")
                nc.sync.dma_start(bvt[:],
                                  bv[:, q * 256 : (q + 1) * 256].bitcast(F32R))
                for sc in range(NKC):
                    ps = pspool.tile([P, 256], F32, tag="vps")
                    for dc in range(ND):
                        nc.tensor.matmul(ps[:],
                                         x_tiles[dc][:, sc * P : (sc + 1) * P],
                                         wslab[:, dc, :],
                                         start=(dc == 0), stop=False)
                    nc.tensor.matmul(ps[:], ones_c[:], bvt[:],
                                     start=False, stop=True)
                    dst = vtiles[sc][:, q * 4 : (q + 1) * 4, 0:DH]
                    nc.scalar.activation(
                        out=dst, in_=ps[:].rearrange("p (h d) -> p h d", h=4),
                        func=AF.Copy)
            return vtiles

        def attention(qt_tiles, kt_producer, v_tiles, mask_tiles, pspools,
                      probsp):
            """qt_tiles: 8 [P, TQ] (2 heads per tile); kt_producer(hp) -> 2
            half tiles [P, 512]; v_tiles: 8 [P, H, DH+1]; mask_tiles or None."""
            scp, ctxps, bcps = pspools
            ctx_tiles = []
            for hp in range(NHP):
                kt_h = kt_producer(hp)
                ctx_t = ctxp.tile([P, TQ], F32R, tag="ctxT")
                ctx_tiles.append(ctx_t)
                qa = qt_tiles[hp][0:64, :]
                qb = qt_tiles[hp][64:128, :]
                psc_a = ctxps.tile([65, TQ], F32, tag="ctxps_a")
                psc_b = ctxps.tile([65, TQ], F32, tag="ctxps_b")
                pending = None
                for kc in range(NKC):
                    kt_t = kt_h[kc // 4]
                    kcol = (kc % 4) * P
                    psA = scp.tile([P, TQ], F32, tag="scA")
                    psB = scp.tile([P, TQ], F32, tag="scB")
                    ka = kt_t[0:64, kcol : kcol + P]
                    kb = kt_t[64:128, kcol : kcol + P]
                    has_mask = mask_tiles is not None
                    nc.tensor.matmul(psA[:], ka, qa, start=True,
                                     stop=not has_mask, tile_position=(0, 0))
                    nc.tensor.matmul(psB[:], kb, qb, start=True,
                                     stop=not has_mask, tile_position=(64, 0))
                    if has_mask:
                        m = mask_tiles[kc]
                        nc.tensor.matmul(psA[:], ident[:], m[:],
                                         start=False, stop=True)
                        nc.tensor.matmul(psB[:], ident[:], m[:],
                                         start=False, stop=True)
                    pA = probsp.tile([P, TQ], F32R, tag="prA")
                    pB = probsp.tile([P, TQ], F32R, tag="prB")
                    nc.scalar.activation(out=pA[:], in_=psA[:], func=AF.Exp,
                                         scale=0.125)
                    nc.scalar.activation(out=pB[:], in_=psB[:], func=AF.Exp,
                                         scale=0.125)
                    if pending is not None:
                        kp, ppA, ppB = pending
                        nc.tensor.matmul(psc_a[:], v_tiles[kp][:, 2 * hp, :],
                                         ppA[:], start=(kp == 0), stop=False)
                        nc.tensor.matmul(psc_b[:], v_tiles[kp][:, 2 * hp + 1, :],
                                         ppB[:], start=(kp == 0), stop=False)
                    pending = (kc, pA, pB)
                kp, ppA, ppB = pending
                nc.tensor.matmul(psc_a[:], v_tiles[kp][:, 2 * hp, :], ppA[:],
                                 start=False, stop=True)
                nc.tensor.matmul(psc_b[:], v_tiles[kp][:, 2 * hp + 1, :], ppB[:],
                                 start=False, stop=True)
                for par, psc in ((0, psc_a), (1, psc_b)):
                    rf = smallp.tile([1, TQ], F32, tag="recf")
                    nc.vector.reciprocal(out=rf[:], in_=psc[64:65, :])
                    rr = smallp.tile([1, TQ], F32R, tag="recr")
                    nc.vector.tensor_copy(out=rr[:], in_=rf[:])
                    psb = bcps.tile([64, TQ], F32, tag="bc")
                    nc.tensor.matmul(psb[:], ones_c[:, 0:64], rr[:],
                                     start=True, stop=True)
                    csb = probsp.tile([64, TQ], F32, tag="csb")
                    nc.scalar.activation(out=csb[:], in_=psc[0:64, :],
                                         func=AF.Copy)
                    nc.vector.tensor_tensor(
                        out=ctx_t[par * 64 : (par + 1) * 64, :], in0=csb[:],
                        in1=psb[:], op=ALU.mult)
            return ctx_tiles

        def layer_norm(pre_tiles, ln_idx, out_pool, out_tag, pools,
                       out_dtype=F32R):
            """LN across the partition (feature) axis of 8 F32R [P, TQ] tiles."""
            tmpp, statps, bcps = pools
            sq_tiles = []
            for dc in range(ND):
                sq = tmpp.tile([P, TQ], F32R, tag="lnsq")
                nc.vector.tensor_tensor(out=sq[:], in0=f32(pre_tiles[dc][:]),
                                        in1=f32(pre_tiles[dc][:]), op=ALU.mult)
                sq_tiles.append(sq)
            ps1 = statps.tile([1, TQ], F32, tag="lns1")
            ps2 = statps.tile([1, TQ], F32, tag="lns2")
            for dc in range(ND):
                nc.tensor.matmul(ps1[:], ones_r[:], pre_tiles[dc][:],
                                 start=(dc == 0), stop=(dc == ND - 1))
            for dc in range(ND):
                nc.tensor.matmul(ps2[:], ones_r[:], sq_tiles[dc][:],
                                 start=(dc == 0), stop=(dc == ND - 1))
            mu = smallp.tile([1, TQ], F32, tag="lnmu")
            nc.vector.tensor_scalar_mul(out=mu[:], in0=ps1[:], scalar1=1.0 / D)
            ex2 = smallp.tile([1, TQ], F32, tag="lnex2")
            nc.vector.tensor_scalar_mul(out=ex2[:], in0=ps2[:], scalar1=1.0 / D)
            var = smallp.tile([1, TQ], F32, tag="lnvar")
            nc.vector.tensor_tensor(out=var[:], in0=mu[:], in1=mu[:], op=ALU.mult)
            nc.vector.tensor_tensor(out=var[:], in0=ex2[:], in1=var[:],
                                    op=ALU.subtract)
            sd = smallp.tile([1, TQ], F32, tag="lnsd")
            nc.scalar.activation(out=sd[:], in_=var[:], func=AF.Sqrt,
                                 bias=eps_t[:], scale=1.0)
            rsf = smallp.tile([1, TQ], F32, tag="lnrsf")
            nc.vector.reciprocal(out=rsf[:], in_=sd[:])
            cmul = smallp.tile([1, TQ], F32, tag="lncm")
            nc.vector.tensor_tensor(out=cmul[:], in0=mu[:], in1=rsf[:],
                                    op=ALU.mult)
            cneg = smallp.tile([1, TQ], F32R, tag="lncn")
            nc.vector.tensor_scalar_mul(out=cneg[:], in0=cmul[:], scalar1=-1.0)
            rsig = smallp.tile([1, TQ], F32R, tag="lnrs")
            nc.vector.tensor_copy(out=rsig[:], in_=rsf[:])
            psa = bcps.tile([P, TQ], F32, tag="lnbca")
            nc.tensor.matmul(psa[:], ones_c[:], rsig[:], start=True, stop=True)
            psc = bcps.tile([P, TQ], F32, tag="lnbcc")
            nc.tensor.matmul(psc[:], ones_c[:], cneg[:], start=True, stop=True)
            out_tiles = []
            for dc in range(ND):
                t1 = tmpp.tile([P, TQ], F32, tag="lnt1")
                nc.vector.tensor_tensor(out=t1[:], in0=f32(pre_tiles[dc][:]),
                                        in1=psa[:], op=ALU.mult)
                t2 = tmpp.tile([P, TQ], F32, tag="lnt2")
                nc.vector.tensor_tensor(out=t2[:], in0=t1[:], in1=psc[:],
                                        op=ALU.add)
                o = out_pool.tile([P, TQ], out_dtype, tag=out_tag)
                nc.vector.tensor_scalar(
                    out=o[:], in0=t2[:],
                    scalar1=lng[:, ln_idx, dc : dc + 1],
                    scalar2=lnb[:, ln_idx, dc : dc + 1],
                    op0=ALU.mult, op1=ALU.add)
                out_tiles.append(o)
            return out_tiles

        def oproj_resid_ln(wo, bo, ctx_tiles, resid_f32_aps, ln_idx, out_tag,
                           stack):
            wpool = stack.enter_context(tc.tile_pool(name="owp", bufs=3))
            pps = stack.enter_context(tc.tile_pool(name="ops", bufs=2,
                                                   space="PSUM"))
            tmpp = stack.enter_context(tc.tile_pool(name="otmp", bufs=3))
            statps = stack.enter_context(
                tc.tile_pool(name="olnst", bufs=1, space="PSUM"))
            bcps = stack.enter_context(
                tc.tile_pool(name="olnbc", bufs=1, space="PSUM"))
            prep = stack.enter_context(tc.tile_pool(name="opre", bufs=8))
            pre_tiles = []
            for jc in range(ND):
                (osb,) = proj_chunk(wo, bo, jc, ctx_tiles, [(0, TQ)], tmpp,
                                    "osb", wpool, pps, out_dtype=F32)
                pre = prep.tile([P, TQ], F32R, tag="pre")
                nc.vector.tensor_tensor(out=pre[:], in0=osb[:],
                                        in1=resid_f32_aps[jc], op=ALU.add)
                pre_tiles.append(pre)
            return layer_norm(pre_tiles, ln_idx, residp, out_tag,
                              (tmpp, statps, bcps))

        # ================= phases =================
        with ExitStack() as ph_t:
            ttp = ph_t.enter_context(tc.tile_pool(name="tTp", bufs=8))
            tT_tiles = load_rows(ttp, tT, T, "tT")

            with ExitStack() as ph_sa:
                qtp = ph_sa.enter_context(tc.tile_pool(name="qtp", bufs=8))
                vxp = ph_sa.enter_context(tc.tile_pool(name="vxp", bufs=8))
                wp = ph_sa.enter_context(tc.tile_pool(name="saw", bufs=3))
                with ExitStack() as ph_p1:
                    pp = ph_p1.enter_context(
                        tc.tile_pool(name="p1ps", bufs=4, space="PSUM"))
                    qt_tiles = []
                    for jc in range(ND):
                        (q_,) = proj_chunk(wq1, bq1, jc, tT_tiles, [(0, TQ)],
                                           qtp, "qt", wp, pp)
                        qt_tiles.append(q_)
                    vwp = ph_p1.enter_context(tc.tile_pool(name="vwp", bufs=2))
                    v_tiles = proj_tokens(wv1, bv1, tT_tiles, vxp, vwp, pp)

                with ExitStack() as ph_a1:
                    mtp = ph_a1.enter_context(tc.tile_pool(name="mtp", bufs=8))
                    m_tiles = load_rows(mtp, mT, TQ, "mt")
                    ktp = ph_a1.enter_context(tc.tile_pool(name="ktp", bufs=3))
                    ktps = ph_a1.enter_context(
                        tc.tile_pool(name="ktps", bufs=1, space="PSUM"))
                    scp = ph_a1.enter_context(
                        tc.tile_pool(name="scps", bufs=2, space="PSUM"))
                    ctxps = ph_a1.enter_context(
                        tc.tile_pool(name="ctxps", bufs=1, space="PSUM"))
                    bcps = ph_a1.enter_context(
                        tc.tile_pool(name="bcps", bufs=1, space="PSUM"))

                    def kt1_producer(hp):
                        return proj_chunk(wk1, bk1, hp, tT_tiles,
                                          [(0, 512), (512, 512)], ktp, "kt",
                                          wp, ktps, pstag="ktps")

                    probsp1 = ph_a1.enter_context(
                        tc.tile_pool(name="probs1", bufs=3))
                    ctx1 = attention(qt_tiles, kt1_producer, v_tiles, m_tiles,
                                     (scp, ctxps, bcps), probsp1)

            with ExitStack() as ph_o1:
                x_tiles = oproj_resid_ln(
                    wo1, bo1, ctx1, [f32(t_[:, 0:TQ]) for t_ in tT_tiles], 0,
                    "resid", ph_o1)

        with ExitStack() as ph_s:
            stp = ph_s.enter_context(tc.tile_pool(name="sTp", bufs=8))
            sT_tiles = load_rows(stp, sT, T, "sT")

            with ExitStack() as ph_ca:
                qtp2 = ph_ca.enter_context(tc.tile_pool(name="qtp2", bufs=8))
                vxp2 = ph_ca.enter_context(tc.tile_pool(name="vxp2", bufs=8))
                wp2 = ph_ca.enter_context(tc.tile_pool(name="caw", bufs=3))
                with ExitStack() as ph_p2:
                    pp = ph_p2.enter_context(
                        tc.tile_pool(name="p2ps", bufs=4, space="PSUM"))
                    qt2 = []
                    for jc in range(ND):
                        (q_,) = proj_chunk(wq2, bq2, jc, x_tiles, [(0, TQ)],
                                           qtp2, "qt2", wp2, pp)
                        qt2.append(q_)
                    vwp2 = ph_p2.enter_context(tc.tile_pool(name="vwp2", bufs=2))
                    v2 = proj_tokens(wv2, bv2, sT_tiles, vxp2, vwp2, pp)

                with ExitStack() as ph_a2:
                    ktp2 = ph_a2.enter_context(tc.tile_pool(name="ktp2", bufs=3))
                    ktps2 = ph_a2.enter_context(
                        tc.tile_pool(name="ktps2", bufs=1, space="PSUM"))
                    scp = ph_a2.enter_context(
                        tc.tile_pool(name="scps2", bufs=2, space="PSUM"))
                    ctxps = ph_a2.enter_context(
                        tc.tile_pool(name="ctxps2", bufs=1, space="PSUM"))
                    bcps = ph_a2.enter_context(
                        tc.tile_pool(name="bcps2", bufs=1, space="PSUM"))

                    def kt2_producer(hp):
                        return proj_chunk(wk2, bk2, hp, sT_tiles,
                                          [(0, 512), (512, 512)], ktp2, "kt2",
                                          wp2, ktps2, pstag="ktps2")

                    probsp2 = ph_a2.enter_context(
                        tc.tile_pool(name="probs2", bufs=3))
                    ctx2 = attention(qt2, kt2_producer, v2, None,
                                     (scp, ctxps, bcps), probsp2)

            with ExitStack() as ph_o2:
                y_tiles = oproj_resid_ln(
                    wo2, bo2, ctx2, [f32(t_[:]) for t_ in x_tiles], 1, "resid",
                    ph_o2)

        # ---- FFN + LN3 ----
        with ExitStack() as ph_f:
            hpool = ph_f.enter_context(tc.tile_pool(name="hp", bufs=12))
            faccp = ph_f.enter_context(tc.tile_pool(name="facc", bufs=8))
            w1p = ph_f.enter_context(tc.tile_pool(name="w1p", bufs=3))
            w2p = ph_f.enter_context(tc.tile_pool(name="w2p", bufs=3))
            b2p = ph_f.enter_context(tc.tile_pool(name="b2p", bufs=8))
            ps1p = ph_f.enter_context(
                tc.tile_pool(name="fps1", bufs=2, space="PSUM"))
            ps2p = ph_f.enter_context(
                tc.tile_pool(name="fps2", bufs=2, space="PSUM"))
            facc = [None] * ND
            b2t = []
            for jc in range(ND):
                bt = b2p.tile([P, 1], F32, tag="b2t")
                nc.sync.dma_start(bt[:], b2[jc * P : (jc + 1) * P, None])
                b2t.append(bt)
            for g in range(4):
                h_tiles = []
                for kl in range(8):
                    k = g * 8 + kl
                    wsl = w1p.tile([P, ND, P], F32R, tag="w1sl")
                    nc.sync.dma_start(
                        wsl[:], w1r[k].rearrange("c p m -> p c m"))
                    bt = w1p.tile([P, 1], F32, tag="b1t")
                    nc.sync.dma_start(bt[:], b1[k * P : (k + 1) * P, None])
                    ps = ps1p.tile([P, TQ], F32, tag="w1ps")
                    for dc in range(ND):
                        nc.tensor.matmul(ps[:], wsl[:, dc, :], y_tiles[dc][:],
                                         start=(dc == 0), stop=(dc == ND - 1))
                    ht = hpool.tile([P, TQ], F32R, tag="h")
                    nc.scalar.activation(out=ht[:], in_=ps[:], func=AF.Relu,
                                         bias=bt[:], scale=1.0)
                    h_tiles.append(ht)
                for jc in range(ND):
                    wsl2 = w2p.tile([P, 8, P], F32R, tag="w2sl")
                    nc.sync.dma_start(
                        wsl2[:],
                        w2r[jc, g * 8 : (g + 1) * 8].rearrange(
                            "c p m -> p c m").bitcast(F32R))
                    ps = ps2p.tile([P, TQ], F32, tag="w2ps")
                    for dcl in range(8):
                        nc.tensor.matmul(ps[:], wsl2[:, dcl, :],
                                         h_tiles[dcl][:],
                                         start=(dcl == 0), stop=(dcl == 7))
                    if g == 0:
                        fa = faccp.tile([P, TQ], F32, tag="fa")
                        nc.scalar.activation(out=fa[:], in_=ps[:],
                                             func=AF.Identity,
                                             bias=b2t[jc][:], scale=1.0)
                        facc[jc] = fa
                    else:
                        nc.vector.tensor_tensor(out=facc[jc][:],
                                                in0=facc[jc][:], in1=ps[:],
                                                op=ALU.add)
            with ExitStack() as ph_l3:
                tmpp = ph_l3.enter_context(tc.tile_pool(name="l3tmp", bufs=3))
                statps = ph_l3.enter_context(
                    tc.tile_pool(name="l3st", bufs=1, space="PSUM"))
                bcps2 = ph_l3.enter_context(
                    tc.tile_pool(name="l3bc", bufs=1, space="PSUM"))
                prep = ph_l3.enter_context(tc.tile_pool(name="l3pre", bufs=8))
                outp = ph_l3.enter_context(tc.tile_pool(name="outp", bufs=3))
                pre3 = []
                for jc in range(ND):
                    pre = prep.tile([P, TQ], F32R, tag="pre3")
                    nc.vector.tensor_tensor(out=pre[:], in0=facc[jc][:],
                                            in1=f32(y_tiles[jc][:]),
                                            op=ALU.add)
                    pre3.append(pre)
                out_tiles = layer_norm(pre3, 2, outp, "out",
                                       (tmpp, statps, bcps2), out_dtype=F32)
                for jc in range(ND):
                    nc.sync.dma_start(outT[jc * P : (jc + 1) * P, :],
                                      out_tiles[jc][:])

    _split_waits(nc)
    return nc


def _host_pack(params):
    def pk(w):
        w = np.ascontiguousarray(np.asarray(w, dtype=np.float32))
        ndc = w.shape[0] // P
        njc = w.shape[1] // P
        return np.ascontiguousarray(
            w.reshape(ndc, P, njc, P).transpose(2, 0, 1, 3))

    p = params
    return {
        "wq1": pk(p["sa_q_W"]), "bq1": np.asarray(p["sa_q_b"], np.float32),
        "wk1": pk(p["sa_k_W"]), "bk1": np.asarray(p["sa_k_b"], np.float32),
        "wv1": np.ascontiguousarray(np.asarray(p["sa_v_W"], np.float32)),
        "bv1": np.asarray(p["sa_v_b"], np.float32).reshape(1, D),
        "wo1": pk(p["sa_o_W"]), "bo1": np.asarray(p["sa_o_b"], np.float32),
        "wq2": pk(p["ca_q_W"]), "bq2": np.asarray(p["ca_q_b"], np.float32),
        "wk2": pk(p["ca_k_W"]), "bk2": np.asarray(p["ca_k_b"], np.float32),
        "wv2": np.ascontiguousarray(np.asarray(p["ca_v_W"], np.float32)),
        "bv2": np.asarray(p["ca_v_b"], np.float32).reshape(1, D),
        "wo2": pk(p["ca_o_W"]), "bo2": np.asarray(p["ca_o_b"], np.float32),
        "w1r": pk(p["ffn_W1"]), "b1": np.asarray(p["ffn_b1"], np.float32),
        "w2r": pk(p["ffn_W2"]), "b2": np.asarray(p["ffn_b2"], np.float32),
        "ln_g": np.stack([np.asarray(p["ln1_g"], np.float32),
                          np.asarray(p["ln2_g"], np.float32),
                          np.asarray(p["ln3_g"], np.float32)]),
        "ln_b": np.stack([np.asarray(p["ln1_b"], np.float32),
                          np.asarray(p["ln2_b"], np.float32),
                          np.asarray(p["ln3_b"], np.float32)]),
        "hident": np.eye(P, dtype=np.float32),
        "hones": np.ones((P, P), dtype=np.float32),
    }


def kernel(target, source, pad_masked, params):
    from concourse.bass_utils import run_bass_kernel_spmd

    target = np.asarray(target, dtype=np.float32)
    source = np.asarray(source, dtype=np.float32)
    mask = np.asarray(pad_masked)

    wshared = _host_pack(params)

    in_maps = []
    for c in range(8):
        b, half = divmod(c, 2)
        q0 = half * TQ
        perm = np.concatenate([np.arange(q0, q0 + TQ),
                               np.arange(0, q0),
                               np.arange(q0 + TQ, T)]).astype(np.int64)
        m = dict(wshared)
        tTb = target[b].T  # [D, T]
        m["tT"] = np.ascontiguousarray(tTb[:, perm])
        m["sT"] = np.ascontiguousarray(source[b].T)
        mrows = (mask[b, q0 : q0 + TQ, :].astype(np.float32)
                 * np.float32(-1e9)).T  # [T keys, TQ]
        m["mT"] = np.ascontiguousarray(mrows[perm, :])
        in_maps.append(m)

    if "nc" not in _CACHE:
        _CACHE["nc"] = _build_decoder()
    res = run_bass_kernel_spmd(_CACHE["nc"], in_maps, core_ids=list(range(8)))

    out = np.empty((B, T, D), dtype=np.float32)
    for c in range(8):
        b, half = divmod(c, 2)
        out[b, half * TQ : (half + 1) * TQ, :] = res.results[c]["outT"].T
    return out
